# revision 73
# baseline (speedup 1.0000x reference)
"""BiAttention (BiDAF-style) Trainium2 kernel — 8-core SPMD, memory-bound.

Contract: kernel(**inputs) takes the FULL tensors
  text [32,8,512,128] f32, query [32,64,128] f32, text_mask [32,8,512],
  query_mask [32,64], w [384], b [1]
and returns attn [32,8,512,512] f32, matching the reference

  w1,w2,w3 = w[:128], w[128:256], w[256:]
  logits[b,m,i,j] = text[b,m,i]·(w3*query[b,j]) + t1[b,m,i] + q2[b,j] + b
  p_q   = softmax_j logits      -> query_attn = p_q @ query
  qlmax = max_j logits          -> p_text = softmax_i qlmax
  text_attn = sum_i p_text*text
  out = concat([text, query_attn, text*query_attn, text*text_attn], -1)

The masks are all ones per the problem spec, so the (1-mask)*VERY_NEG term
is identically zero and bias b cancels inside softmax_j.  t1 is FOLDED into
the cross matmul on the host (every wq3 column gets +w1), so eT = exp(full
logits): the per-i t1 shift cancels in softmax_j (qa unchanged) and
max_j eT = exp(qlmax) directly — no separate t1 row or exp(t1) multiply.

v4 — HBM-traffic-minimized (16.9 MB/core vs 41.9 in the f32 baseline);
measured 62.8us/core (TimelineSim; hardware rel err 4.2e-3) vs 121.4us:
  * output block 0 is a verbatim copy of the input `text`; the host fills it
    during unshard; the device stores only [query_attn, text*query_attn,
    text*text_attn] in fp16 (upcast to f32 on host).
  * text is loaded bf16 with a paired-row interleave (partition p holds rows
    {256t+2p, 256t+2p+1}) keeping every DMA descriptor >= 512 B contiguous
    on BOTH the DRAM and SBUF side (the DGE halves throughput below 512 B).
  * all matmuls bf16 (1 PE col/cycle): text transposes, cross [64,512], eT
    transposes (64-col blocks, naturally PSUM-aligned), attnu, text_attn.
  * separate 1-col matmuls against a host-packed ones column give the
    softmax_j denominators Z(i) per-partition (one [128,4] reciprocal + one
    512-wide DVE mul normalizes all of qa).
  * text_attn: stride-0-broadcast stationary (every PE column = etq) makes
    the weighted-sum matmul emit its result broadcast across all 128
    partitions; Zt rides the same trick; one ACT scalar-mul normalizes.
  * steady state is co-limited by DVE(~1.5us/unit: qlmax max-reduce, qa,
    col shares) / Pool(col2+col3 shares) / ACT(exp + textd copy 1.5us) /
    DMA(1.46us) — engine split tuned via CFG against TimelineSim.
  * pipelining: text prep (load/transpose/copy) issued one unit ahead;
    tabcN+col3+store deferred one unit (defer_ta) so the in-order ACT queue
    never head-blocks on a fresh normalizer; per-batch params prefetched on
    spread-out units; gb0 params via Pool SWDGE to keep the startup HWDGE
    free; tail units split stores/col-muls across engines.

Sharding: batch B=32 data-parallel across 8 NeuronCores (BLOC=4 per core),
32 (b,m) units per core, no collectives.  Host precomputes O(query)-sized
helpers only (packed into one bf16 tile per batch + tiny f32 q2).

Toolchain note: walrus in this container encodes ONE sync-wait per
instruction; _split_multi_waits() legalizes the Tile-emitted program,
keeping the latest-firing semaphore on the instruction itself.
"""

import os
import sys

for _p in ("/opt/trn_rl_repo", "/root/.axon_site/_ro/trn_rl_repo"):
    if os.path.isdir(_p) and _p not in sys.path:
        sys.path.insert(0, _p)

import numpy as np
import ml_dtypes

import concourse.bass as bass
import concourse.tile as tile
from concourse import mybir
from concourse.bass_utils import run_bass_kernel_spmd
from concourse.masks import make_identity

NCORES = 8
B, M, JX, JQ, D = 32, 8, 512, 64, 128
BLOC = B // NCORES          # batches per core
NT = JX // 128              # 128-col i-blocks per (b,m)
NTH = NT // 2               # paired-row DMA t-blocks
F32 = mybir.dt.float32
BF16 = mybir.dt.bfloat16
FP16 = mybir.dt.float16


def _split_multi_waits(nc):
    """walrus encodes one sync-wait per instruction; Tile may attach several.
    Split the extras into standalone EventSemaphore (sequencer wait)
    instructions placed directly before the instruction on the same engine.

    Standalone waits BLOCK the sequencer, while the wait carried on the
    instruction itself parks in the engine wait-queue (sequencer keeps
    dispatching).  So keep the LATEST-firing semaphore on the instruction:
    sort waits by the program position of each semaphore's last updater and
    emit the earlier ones (usually already satisfied) as standalone."""
    n = 0
    for fn in nc.m.functions:
        for bb in fn.blocks:
            # map semaphore id -> last program position that updates it
            last_upd = {}
            for idx, inst in enumerate(bb.instructions):
                si = inst.sync_info
                if si is not None:
                    for upd in si.on_update:
                        last_upd[getattr(upd, "id", None)] = idx
            out = []
            for inst in bb.instructions:
                si = inst.sync_info
                if si is not None and si.on_wait and len(si.on_wait) > 1:
                    waits = sorted(
                        si.on_wait,
                        key=lambda w: last_upd.get(getattr(w, "id", None), -1))
                    for k, w in enumerate(waits[:-1]):
                        out.append(mybir.InstEventSemaphore(
                            name=f"{inst.name}-sw{k}",
                            engine=inst.engine,
                            ins=[], outs=[],
                            sync_info=mybir.SyncInfo(on_wait=[w], on_update=[]),
                        ))
                        n += 1
                    inst.sync_info = mybir.SyncInfo(
                        on_wait=[waits[-1]], on_update=list(si.on_update))
                out.append(inst)
            bb.instructions = out
    return n


def _bcast(ap, reps, axis):
    """Stride-0 broadcast AP: insert [0, reps] at `axis` of ap's dims."""
    a = [list(d) for d in ap.ap]
    a.insert(axis, [0, reps])
    return bass.AP(tensor=ap.tensor, offset=ap.offset, ap=a)


def _col_bcast(ap_col, reps):
    """[128,1] column AP -> [128, reps] stride-0 stationary broadcast."""
    return bass.AP(tensor=ap_col.tensor, offset=ap_col.offset,
                   ap=[list(ap_col.ap[0]), [0, reps]])


CFG = dict(
    eng_textd="act",    # transposed-text PSUM->SBUF copy: act|dve
    eng_tabc="act",     # tabc normalize+cast: act|dve
    qa_merge=True,      # Z cols via separate tiny matmuls; single qa op
    merge_recip=False,  # one reciprocal for [Zt | Z(i)] vs two
    col2_pool_u=2,      # how many of the 4 col2 u-blocks run on Pool
    col3_pool_u=2,      # how many of the 4 col3 u-blocks run on Pool
    q_tin="sync", q_out="sync", q_small="sync", q_first="gpsimd",
    ptext=16, ptextd=3, pet=4, po123=16, psmall=12, ptabc=4,
    ttp=2, cross=1, etr=1, attnu=2, tabc=2,
    split_in=1, split_out=1, tail_split=2, tail_cols_dve=3, defer_ta=1,
    q_tin0="sync", tail_qa_act=0, defer_tail_off=0, pair_ql=0,
    split_exp=1, tail_prep=0, head_textd_dve=0,
)


def _build_program():
    nc = bass.Bass()
    t_text = nc.dram_tensor("text", [BLOC, M, JX, D], BF16, kind="ExternalInput")
    # packed per-batch params: cols [0:65]=wq3aug [128 rows], [65:194]=qnaug
    # [rows 0:64 = [qn | ones]]
    t_pk = nc.dram_tensor("packed", [BLOC, D, D + JQ + 3], BF16, kind="ExternalInput")
    t_q2 = nc.dram_tensor("q2aug", [BLOC, JQ, 1], F32, kind="ExternalInput")
    t_out = nc.dram_tensor("out", [BLOC, M, JX, 3 * D], FP16, kind="ExternalOutput")

    with tile.TileContext(nc) as tc:
        import contextlib
        ctx = contextlib.ExitStack()
        with ctx:
            singles = ctx.enter_context(tc.tile_pool(name="singles", bufs=1))
            perb = ctx.enter_context(tc.tile_pool(name="perb", bufs=2))
            ptext = ctx.enter_context(tc.tile_pool(name="ptext", bufs=CFG["ptext"]))
            ptextd = ctx.enter_context(tc.tile_pool(name="ptextd", bufs=CFG["ptextd"]))
            pet = ctx.enter_context(tc.tile_pool(name="pet", bufs=CFG["pet"]))
            po123 = ctx.enter_context(tc.tile_pool(name="po123", bufs=CFG["po123"]))
            psmall = ctx.enter_context(tc.tile_pool(name="psmall", bufs=CFG["psmall"]))
            ptabc = ctx.enter_context(tc.tile_pool(name="ptabc", bufs=CFG["ptabc"]))
            ps_ttp = ctx.enter_context(tc.tile_pool(name="ps_ttp", bufs=CFG["ttp"], space="PSUM"))
            ps_cross = ctx.enter_context(tc.tile_pool(name="ps_cross", bufs=CFG["cross"], space="PSUM"))
            ps_etr = ctx.enter_context(tc.tile_pool(name="ps_etr", bufs=CFG["etr"], space="PSUM"))
            ps_tabc = ctx.enter_context(tc.tile_pool(name="ps_tabc", bufs=CFG["tabc"], space="PSUM"))
            ps_attnu = ctx.enter_context(tc.tile_pool(name="ps_attnu", bufs=CFG["attnu"], space="PSUM"))

            # issue the very first text load before any constant setup so the
            # DMA engines start immediately
            first_text = ptext.tile([128, NT * D], BF16, tag="text")
            _fsrc = t_text[0, 0].rearrange("(t p k) d -> p t k d", p=128, k=2)
            getattr(nc, CFG["q_tin0"]).dma_start(
                out=first_text.rearrange("p (t k d) -> p t k d", t=NTH, k=2),
                in_=_fsrc)

            identb = singles.tile([128, 128], BF16)
            make_identity(nc, identb)
            identb64 = identb[0:JQ, 0:JQ]
            ones128 = singles.tile([128, 128], BF16)
            nc.vector.memset(ones128, 1.0)

            def prep_text(gb, m):
                """Load + transpose + PSUM->SBUF copy of one text unit.
                Issued one unit AHEAD of the consuming compute so the ACT
                textd copy does not sit between exp(n) and cross(n+1) on the
                in-order ACT queue (that cycle gates the whole pipeline)."""
                # partition p, block u=2t+k holds DRAM row i=256t+2p+k
                if gb == 0 and m == 0:
                    text_il = first_text
                else:
                    text_il = ptext.tile([128, NT * D], BF16, tag="text")
                    src = t_text[gb, m].rearrange(
                        "(t p k) d -> p t k d", p=128, k=2)
                    dst = text_il.rearrange(
                        "p (t k d) -> p t k d", t=NTH, k=2)
                    nsi = CFG["split_in"]
                    for h in range(nsi):
                        hh = NTH // nsi
                        getattr(nc, CFG["q_tin"]).dma_start(
                            out=dst[:, h * hh:(h + 1) * hh],
                            in_=src[:, h * hh:(h + 1) * hh])
                ttp = ps_ttp.tile([128, JX], BF16, tag="ttp")
                for u in range(NT):
                    nc.tensor.transpose(
                        ttp[:, u * 128:(u + 1) * 128],
                        text_il[:, u * D:(u + 1) * D], identb)
                textd = ptextd.tile([128, JX], BF16, tag="textd")
                # first units: DVE is idle during the ramp while the ACT
                # queue position gates the first cross->exp
                if gb * M + m < CFG["head_textd_dve"]:
                    nc.vector.tensor_copy(textd, ttp)
                elif CFG["eng_textd"] == "act":
                    nc.scalar.copy(out=textd, in_=ttp)
                else:
                    nc.vector.tensor_copy(textd, ttp)
                return text_il, textd

            def load_params(gb, queue=None, part=None):
                qd = getattr(nc, queue or CFG["q_small"])
                if part in (None, 0):
                    pk_sb = perb.tile([D, D + JQ + 3], BF16, tag="pk")
                    qd.dma_start(out=pk_sb, in_=t_pk[gb])
                    load_params.pk = pk_sb
                if part in (None, 1):
                    q2_sb = perb.tile([JQ, 1], F32, tag="q2")
                    qd.dma_start(out=q2_sb, in_=t_q2[gb])
                    load_params.q2 = q2_sb
                return load_params.pk, load_params.q2

            def taupart(tabu, text_il, etq_ap, rzt=None):
                """text_attn broadcast (every PE column = etq) + Zt bcast
                + reciprocal.  etq_ap is a [128, NT] AP of exp(qlmax)."""
                for u in range(NT):
                    nc.tensor.matmul(
                        tabu[:, 0:D],
                        _col_bcast(etq_ap[:, u:u + 1], 128),
                        text_il[:, u * D:(u + 1) * D],
                        start=(u == 0), stop=(u == NT - 1))
                if rzt is None:
                    for u in range(NT):
                        nc.tensor.matmul(
                            tabu[:, D:D + 1],
                            _col_bcast(etq_ap[:, u:u + 1], 128),
                            ones128[:, 0:1],
                            start=(u == 0), stop=(u == NT - 1))
                    rzt = psmall.tile([128, 1], F32, tag="rzt")
                    nc.vector.reciprocal(out=rzt, in_=tabu[:, D:D + 1])
                return rzt

            pending = [prep_text(0, 0)]
            prep_next = [1]
            pending_backend = None
            pair_stash = []
            etr_pair = None
            # gb0 params ride the idle SP queue at startup so the ACT
            # sequencer is free for the first textd copy
            params = load_params(0, queue=CFG["q_first"])
            for gb in range(BLOC):
                pk_sb, q2_sb = params
                # w1 is folded into every wq3 column on the host, so cross
                # already includes t1 and eT = exp(full logits); the per-i
                # t1 shift cancels in softmax_j and max_j eT = exp(qlmax)
                wq3_sb = pk_sb[:, 0:JQ]
                qn_sb = pk_sb[0:JQ, JQ: JQ + D + 1]

                for m in range(M):
                    # prefetch next batch's params mid-batch so the first
                    # unit of gb+1 never stalls on them; the two triggers go
                    # in different units so their sequencer cost spreads out
                    if gb + 1 < BLOC:
                        if m == 3:
                            load_params(gb + 1, part=0)
                        elif m == 5:
                            params = load_params(gb + 1, part=1)
                    unit = gb * M + m
                    text_il, textd = pending.pop(0)
                    text3 = text_il.rearrange("p (u d) -> p u d", d=D)

                    # ---- crossT_aug = [w3q|w1].T @ text_d  [65, 512] ----
                    cross = ps_cross.tile([JQ, JX], F32, tag="cross")
                    nc.tensor.matmul(cross, wq3_sb, textd, start=True, stop=True)

                    # ---- eT = exp(cross + q2) (row 64 = exp(t1)) ----
                    # split into column halves: same total ACT time (init is
                    # width-independent) but downstream consumers of half 0
                    # start ~400ns earlier
                    eT = pet.tile([JQ, JX], BF16, tag="eT")
                    nse = CFG["split_exp"]
                    he = JX // nse
                    for h in range(nse):
                        nc.scalar.activation(
                            out=eT[:, h * he:(h + 1) * he],
                            in_=cross[:, h * he:(h + 1) * he],
                            func=mybir.ActivationFunctionType.Exp,
                            bias=q2_sb[:, 0:1], scale=1.0)

                    # prep the NEXT unit now: its ACT copy lands after exp(n)
                    # and its PE transposes after cross(n), so neither blocks
                    # the cross->exp critical cycle.  Near the tail, prep
                    # additional units ahead so the final exps bunch early.
                    ahead = 1
                    if BLOC * M - unit <= CFG["tail_prep"]:
                        ahead = 2
                    while len(pending) < ahead and prep_next[0] < BLOC * M:
                        nxt = prep_next[0]
                        prep_next[0] += 1
                        pending.append(prep_text(nxt // M, nxt % M))

                    # ---- transpose eT slices -> etr [128, 4*64] ----
                    JB = JQ
                    if CFG["pair_ql"]:
                        if unit % 2 == 0:
                            etr_pair = ps_etr.tile(
                                [128, 2 * NT * JB], BF16, tag="etr")
                        etr = etr_pair[:, (unit % 2) * NT * JB:
                                       (unit % 2 + 1) * NT * JB]
                    else:
                        etr_t = ps_etr.tile([128, NT * JB], BF16, tag="etr")
                        etr = etr_t[:, :]
                    for u in range(NT):
                        nc.tensor.transpose(
                            etr[:, u * JB:u * JB + JQ],
                            eT[:, u * 128:(u + 1) * 128], identb64)

                    # ---- qlmax path: etq = exp(qlmax) = G * exp(t1) ----
                    # (pair mode: one reduce+mul covers TWO units' etr,
                    # halving the per-op DVE init overhead)
                    etq = etq_pair = None
                    if not CFG["pair_ql"]:
                        etr_blk = etr.rearrange("p (u j) -> p u j", j=JB)
                        etq = psmall.tile([128, NT], BF16, tag="etq")
                        nc.vector.tensor_reduce(
                            out=etq, in_=etr_blk,
                            axis=mybir.AxisListType.X, op=mybir.AluOpType.max)
                    elif unit % 2 == 1:
                        pblk = etr_pair[:, :].rearrange(
                            "p (w j) -> p w j", j=JB)
                        etq_pair = psmall.tile([128, 2 * NT], BF16, tag="etq")
                        nc.vector.tensor_reduce(
                            out=etq_pair, in_=pblk,
                            axis=mybir.AxisListType.X, op=mybir.AluOpType.max)

                    # ---- attnu = eT[0:64].T @ qn; qa = attnu*rq ----
                    # tabu regions: [0:D] text_attn bcast, [D] Zt bcast,
                    # [D+1:D+1+NT] attnu softmax denominators Z(i)
                    tabu = ps_tabc.tile([128, D + 1 + NT], F32, tag="tabu")
                    o123 = po123.tile([128, NT, 3 * D], FP16, tag="o123")
                    if CFG["qa_merge"]:
                        onesq = pk_sb[0:JQ, JQ + D:JQ + 1 + D]
                        attnu = ps_attnu.tile([128, NT * D], F32, tag="attnu")
                        for u in range(NT):
                            nc.tensor.matmul(
                                attnu[:, u * D:(u + 1) * D],
                                eT[0:JQ, u * 128:(u + 1) * 128],
                                qn_sb[:, 0:D], start=True, stop=True)
                        for u in range(NT):
                            nc.tensor.matmul(
                                tabu[:, D + 1 + u:D + 2 + u],
                                eT[0:JQ, u * 128:(u + 1) * 128],
                                onesq, start=True, stop=True)
                        if CFG["merge_recip"]:
                            # Zt via accumulating 1-column matmuls (etq col
                            # bcast stationary, ones col moving) issued here
                            # so ONE reciprocal covers [Zt | Z(i) x4]
                            for u in range(NT):
                                nc.tensor.matmul(
                                    tabu[:, D:D + 1],
                                    _col_bcast(etq[:, u:u + 1], 128),
                                    ones128[:, 0:1],
                                    start=(u == 0), stop=(u == NT - 1))
                            rqz = psmall.tile([128, NT + 1], F32, tag="rqz")
                            nc.vector.reciprocal(
                                out=rqz, in_=tabu[:, D:D + 1 + NT])
                            rq = rqz[:, 1:NT + 1]
                            rzt = rqz[:, 0:1]
                        else:
                            rq = psmall.tile([128, NT], F32, tag="rq")
                            nc.vector.reciprocal(
                                out=rq, in_=tabu[:, D + 1:D + 1 + NT])
                            rzt = None
                        if BLOC * M - unit <= CFG["tail_qa_act"]:
                            for u in range(NT):
                                nc.scalar.mul(
                                    out=o123[:, u, 0:D],
                                    in_=attnu[:, u * D:(u + 1) * D],
                                    mul=rq[:, u:u + 1])
                        else:
                            nc.vector.tensor_tensor(
                                out=o123[:, :, 0:D],
                                in0=attnu.rearrange("p (u d) -> p u d", d=D),
                                in1=_bcast(rq, D, 2),
                                op=mybir.AluOpType.mult)
                    else:
                        raise NotImplementedError("qa_merge=False removed")

                    if not CFG["pair_ql"]:
                        rzt = taupart(tabu, text_il, etq, rzt)
                    # ---- col2 = text*qa (qa is fresh; stays in-unit) ----
                    tail = BLOC * M - unit <= CFG["tail_cols_dve"]
                    dve_u = NT if tail else NT - CFG["col2_pool_u"]
                    spans = (((nc.vector, 0, 2), (nc.gpsimd, 2, NT)) if tail
                             else ((nc.vector, 0, dve_u),
                                   (nc.gpsimd, dve_u, NT)))
                    for eng, u0, u1 in spans:
                        if u1 <= u0:
                            continue
                        eng.tensor_mul(
                            o123[:, u0:u1, D:2 * D],
                            text3[:, u0:u1, :], o123[:, u0:u1, 0:D])

                    def backend(rzt, gb=gb, m=m, unit=unit, tabu=tabu,
                                o123=o123, text3=text3):
                        """tabcN + col3 + store; optionally run one unit
                        late so the ACT tabcN never head-blocks on a fresh
                        rzt and the store order stays compute-paced."""
                        tabc = ptabc.tile([128, D], BF16, tag="tabc")
                        if CFG["eng_tabc"] == "act":
                            nc.scalar.mul(out=tabc, in_=tabu[:, 0:D],
                                          mul=rzt)
                        else:
                            nc.vector.tensor_scalar_mul(
                                out=tabc, in0=tabu[:, 0:D], scalar1=rzt)
                        tail = BLOC * M - unit <= CFG["tail_cols_dve"]
                        dve_u = NT if tail else NT - CFG["col3_pool_u"]
                        spans = (((nc.vector, 0, 2), (nc.gpsimd, 2, NT))
                                 if tail else ((nc.vector, 0, dve_u),
                                               (nc.gpsimd, dve_u, NT)))
                        for eng, u0, u1 in spans:
                            if u1 <= u0:
                                continue
                            eng.tensor_mul(
                                o123[:, u0:u1, 2 * D:3 * D],
                                text3[:, u0:u1, :],
                                _bcast(tabc[:, :], u1 - u0, 1))
                        nsp = CFG["split_out"]
                        if BLOC * M - unit <= CFG["tail_split"]:
                            nsp = max(nsp, 2)
                        ht = NT // nsp
                        dst4 = t_out[gb, m].rearrange(
                            "(t p k) c -> p t k c", p=128, k=2)
                        o1234 = o123[:, :, :].rearrange(
                            "p (t k) c -> p t k c", k=2)
                        for h in range(nsp):
                            ts0, ts1 = h * ht, (h + 1) * ht
                            getattr(nc, CFG["q_out"]).dma_start(
                                out=dst4[:, ts0 // 2:ts1 // 2],
                                in_=o1234[:, ts0 // 2:ts1 // 2])

                    if CFG["pair_ql"]:
                        pair_stash.append((tabu, text_il, backend))
                        if unit % 2 == 1:
                            for i, (tb, til, bk) in enumerate(pair_stash):
                                bk(taupart(
                                    tb, til,
                                    etq_pair[:, i * NT:(i + 1) * NT]))
                            pair_stash = []
                    elif (CFG["defer_ta"]
                            and BLOC * M - unit > CFG["defer_tail_off"]):
                        if pending_backend is not None:
                            pending_backend()
                        pending_backend = (lambda r=rzt, b=backend: b(r))
                    else:
                        if pending_backend is not None:
                            pending_backend()
                            pending_backend = None
                        backend(rzt)

            if pending_backend is not None:
                pending_backend()

    _split_multi_waits(nc)
    return nc


_NC_CACHE = {}


def _get_nc():
    if "nc" not in _NC_CACHE:
        _NC_CACHE["nc"] = _build_program()
    return _NC_CACHE["nc"]


def _make_in_maps(text, query, w):
    w1, w2, w3 = w[:D], w[D:2 * D], w[2 * D:]
    in_maps = []
    for c in range(NCORES):
        sl = slice(c * BLOC, (c + 1) * BLOC)
        q = query[sl]                                    # [BLOC, 64, 128]
        q2 = np.einsum("bjd,d->bj", q, w2)[:, :, None]
        # packed [D, 64 + 129 + 2]: [0:64] = w3*q_j + w1 (w1 folded into
        # every column so cross includes t1); rows 0:64 of [64:193] =
        # [qn | ones]; trailing cols pad
        pk = np.zeros((BLOC, D, D + JQ + 3), np.float32)
        pk[:, :, 0:JQ] = (np.einsum("bjd->bdj", q * w3[None, None, :])
                          + w1[None, :, None])
        pk[:, 0:JQ, JQ:JQ + D] = q
        pk[:, 0:JQ, JQ + D] = 1.0
        m = {
            "text": np.ascontiguousarray(text[sl]).astype(ml_dtypes.bfloat16),
            "packed": np.ascontiguousarray(pk).astype(ml_dtypes.bfloat16),
            "q2aug": np.ascontiguousarray(q2, dtype=np.float32),
        }
        in_maps.append(m)
    return in_maps


def kernel(text, query, text_mask, query_mask, w, b, _want_results=False):
    text = np.asarray(text, dtype=np.float32)
    query = np.asarray(query, dtype=np.float32)
    w = np.asarray(w, dtype=np.float32)
    nc = _get_nc()
    in_maps = _make_in_maps(text, query, w)
    res = run_bass_kernel_spmd(nc, in_maps, core_ids=list(range(NCORES)))
    out = np.empty((B, M, JX, 4 * D), dtype=np.float32)
    out[..., 0:D] = text
    for c in range(NCORES):
        out[c * BLOC:(c + 1) * BLOC, ..., D:] = res.results[c]["out"]
    if _want_results:
        return out, res
    return out


# revision 74
# speedup vs baseline: 1.0021x; 1.0021x over previous
"""BiAttention (BiDAF-style) Trainium2 kernel — 8-core SPMD, memory-bound.

Contract: kernel(**inputs) takes the FULL tensors
  text [32,8,512,128] f32, query [32,64,128] f32, text_mask [32,8,512],
  query_mask [32,64], w [384], b [1]
and returns attn [32,8,512,512] f32, matching the reference

  w1,w2,w3 = w[:128], w[128:256], w[256:]
  logits[b,m,i,j] = text[b,m,i]·(w3*query[b,j]) + t1[b,m,i] + q2[b,j] + b
  p_q   = softmax_j logits      -> query_attn = p_q @ query
  qlmax = max_j logits          -> p_text = softmax_i qlmax
  text_attn = sum_i p_text*text
  out = concat([text, query_attn, text*query_attn, text*text_attn], -1)

The masks are all ones per the problem spec, so the (1-mask)*VERY_NEG term
is identically zero and bias b cancels inside softmax_j.  t1 is FOLDED into
the cross matmul on the host (every wq3 column gets +w1), so eT = exp(full
logits): the per-i t1 shift cancels in softmax_j (qa unchanged) and
max_j eT = exp(qlmax) directly — no separate t1 row or exp(t1) multiply.

v4 — HBM-traffic-minimized (16.9 MB/core vs 41.9 in the f32 baseline);
measured 62.8us/core (TimelineSim; hardware rel err 4.2e-3) vs 121.4us:
  * output block 0 is a verbatim copy of the input `text`; the host fills it
    during unshard; the device stores only [query_attn, text*query_attn,
    text*text_attn] in fp16 (upcast to f32 on host).
  * text is loaded bf16 with a paired-row interleave (partition p holds rows
    {256t+2p, 256t+2p+1}) keeping every DMA descriptor >= 512 B contiguous
    on BOTH the DRAM and SBUF side (the DGE halves throughput below 512 B).
  * all matmuls bf16 (1 PE col/cycle): text transposes, cross [64,512], eT
    transposes (64-col blocks, naturally PSUM-aligned), attnu, text_attn.
  * separate 1-col matmuls against a host-packed ones column give the
    softmax_j denominators Z(i) per-partition (one [128,4] reciprocal + one
    512-wide DVE mul normalizes all of qa).
  * text_attn: stride-0-broadcast stationary (every PE column = etq) makes
    the weighted-sum matmul emit its result broadcast across all 128
    partitions; Zt rides the same trick; one ACT scalar-mul normalizes.
  * steady state is co-limited by DVE(~1.5us/unit: qlmax max-reduce, qa,
    col shares) / Pool(col2+col3 shares) / ACT(exp + textd copy 1.5us) /
    DMA(1.46us) — engine split tuned via CFG against TimelineSim.
  * pipelining: text prep (load/transpose/copy) issued one unit ahead;
    tabcN+col3+store deferred one unit (defer_ta) so the in-order ACT queue
    never head-blocks on a fresh normalizer; per-batch params prefetched on
    spread-out units; gb0 params via Pool SWDGE to keep the startup HWDGE
    free; tail units split stores/col-muls across engines.

Sharding: batch B=32 data-parallel across 8 NeuronCores (BLOC=4 per core),
32 (b,m) units per core, no collectives.  Host precomputes O(query)-sized
helpers only (packed into one bf16 tile per batch + tiny f32 q2).

Toolchain note: walrus in this container encodes ONE sync-wait per
instruction; _split_multi_waits() legalizes the Tile-emitted program,
keeping the latest-firing semaphore on the instruction itself.
"""

import os
import sys

for _p in ("/opt/trn_rl_repo", "/root/.axon_site/_ro/trn_rl_repo"):
    if os.path.isdir(_p) and _p not in sys.path:
        sys.path.insert(0, _p)

import numpy as np
import ml_dtypes

import concourse.bass as bass
import concourse.tile as tile
from concourse import mybir
from concourse.bass_utils import run_bass_kernel_spmd
from concourse.masks import make_identity

NCORES = 8
B, M, JX, JQ, D = 32, 8, 512, 64, 128
BLOC = B // NCORES          # batches per core
NT = JX // 128              # 128-col i-blocks per (b,m)
NTH = NT // 2               # paired-row DMA t-blocks
F32 = mybir.dt.float32
BF16 = mybir.dt.bfloat16
FP16 = mybir.dt.float16


def _split_multi_waits(nc):
    """walrus encodes one sync-wait per instruction; Tile may attach several.
    Split the extras into standalone EventSemaphore (sequencer wait)
    instructions placed directly before the instruction on the same engine.

    Standalone waits BLOCK the sequencer, while the wait carried on the
    instruction itself parks in the engine wait-queue (sequencer keeps
    dispatching).  So keep the LATEST-firing semaphore on the instruction:
    sort waits by the program position of each semaphore's last updater and
    emit the earlier ones (usually already satisfied) as standalone."""
    n = 0
    for fn in nc.m.functions:
        for bb in fn.blocks:
            # map semaphore id -> last program position that updates it
            last_upd = {}
            for idx, inst in enumerate(bb.instructions):
                si = inst.sync_info
                if si is not None:
                    for upd in si.on_update:
                        last_upd[getattr(upd, "id", None)] = idx
            out = []
            for inst in bb.instructions:
                si = inst.sync_info
                if si is not None and si.on_wait and len(si.on_wait) > 1:
                    waits = sorted(
                        si.on_wait,
                        key=lambda w: last_upd.get(getattr(w, "id", None), -1))
                    for k, w in enumerate(waits[:-1]):
                        out.append(mybir.InstEventSemaphore(
                            name=f"{inst.name}-sw{k}",
                            engine=inst.engine,
                            ins=[], outs=[],
                            sync_info=mybir.SyncInfo(on_wait=[w], on_update=[]),
                        ))
                        n += 1
                    inst.sync_info = mybir.SyncInfo(
                        on_wait=[waits[-1]], on_update=list(si.on_update))
                out.append(inst)
            bb.instructions = out
    return n


def _bcast(ap, reps, axis):
    """Stride-0 broadcast AP: insert [0, reps] at `axis` of ap's dims."""
    a = [list(d) for d in ap.ap]
    a.insert(axis, [0, reps])
    return bass.AP(tensor=ap.tensor, offset=ap.offset, ap=a)


def _col_bcast(ap_col, reps):
    """[128,1] column AP -> [128, reps] stride-0 stationary broadcast."""
    return bass.AP(tensor=ap_col.tensor, offset=ap_col.offset,
                   ap=[list(ap_col.ap[0]), [0, reps]])


CFG = dict(
    eng_textd="act",    # transposed-text PSUM->SBUF copy: act|dve
    eng_tabc="act",     # tabc normalize+cast: act|dve
    qa_merge=True,      # Z cols via separate tiny matmuls; single qa op
    merge_recip=False,  # one reciprocal for [Zt | Z(i)] vs two
    col2_pool_u=2,      # how many of the 4 col2 u-blocks run on Pool
    col3_pool_u=2,      # how many of the 4 col3 u-blocks run on Pool
    q_tin="sync", q_out="sync", q_small="sync", q_first="gpsimd",
    ptext=16, ptextd=3, pet=4, po123=16, psmall=12, ptabc=4,
    ttp=2, cross=1, etr=1, attnu=2, tabc=2,
    split_in=1, split_out=1, tail_split=1, tail_cols_dve=3, defer_ta=1,
    q_tin0="sync", tail_qa_act=0, defer_tail_off=0, pair_ql=0,
    split_exp=1, tail_prep=0, head_textd_dve=0,
)


def _build_program():
    nc = bass.Bass()
    t_text = nc.dram_tensor("text", [BLOC, M, JX, D], BF16, kind="ExternalInput")
    # packed per-batch params: cols [0:65]=wq3aug [128 rows], [65:194]=qnaug
    # [rows 0:64 = [qn | ones]]
    t_pk = nc.dram_tensor("packed", [BLOC, D, D + JQ + 3], BF16, kind="ExternalInput")
    t_q2 = nc.dram_tensor("q2aug", [BLOC, JQ, 1], F32, kind="ExternalInput")
    t_out = nc.dram_tensor("out", [BLOC, M, JX, 3 * D], FP16, kind="ExternalOutput")

    with tile.TileContext(nc) as tc:
        import contextlib
        ctx = contextlib.ExitStack()
        with ctx:
            singles = ctx.enter_context(tc.tile_pool(name="singles", bufs=1))
            perb = ctx.enter_context(tc.tile_pool(name="perb", bufs=2))
            ptext = ctx.enter_context(tc.tile_pool(name="ptext", bufs=CFG["ptext"]))
            ptextd = ctx.enter_context(tc.tile_pool(name="ptextd", bufs=CFG["ptextd"]))
            pet = ctx.enter_context(tc.tile_pool(name="pet", bufs=CFG["pet"]))
            po123 = ctx.enter_context(tc.tile_pool(name="po123", bufs=CFG["po123"]))
            psmall = ctx.enter_context(tc.tile_pool(name="psmall", bufs=CFG["psmall"]))
            ptabc = ctx.enter_context(tc.tile_pool(name="ptabc", bufs=CFG["ptabc"]))
            ps_ttp = ctx.enter_context(tc.tile_pool(name="ps_ttp", bufs=CFG["ttp"], space="PSUM"))
            ps_cross = ctx.enter_context(tc.tile_pool(name="ps_cross", bufs=CFG["cross"], space="PSUM"))
            ps_etr = ctx.enter_context(tc.tile_pool(name="ps_etr", bufs=CFG["etr"], space="PSUM"))
            ps_tabc = ctx.enter_context(tc.tile_pool(name="ps_tabc", bufs=CFG["tabc"], space="PSUM"))
            ps_attnu = ctx.enter_context(tc.tile_pool(name="ps_attnu", bufs=CFG["attnu"], space="PSUM"))

            # issue the very first text load before any constant setup so the
            # DMA engines start immediately
            first_text = ptext.tile([128, NT * D], BF16, tag="text")
            _fsrc = t_text[0, 0].rearrange("(t p k) d -> p t k d", p=128, k=2)
            getattr(nc, CFG["q_tin0"]).dma_start(
                out=first_text.rearrange("p (t k d) -> p t k d", t=NTH, k=2),
                in_=_fsrc)

            identb = singles.tile([128, 128], BF16)
            make_identity(nc, identb)
            identb64 = identb[0:JQ, 0:JQ]
            ones128 = singles.tile([128, 128], BF16)
            nc.vector.memset(ones128, 1.0)

            def prep_text(gb, m):
                """Load + transpose + PSUM->SBUF copy of one text unit.
                Issued one unit AHEAD of the consuming compute so the ACT
                textd copy does not sit between exp(n) and cross(n+1) on the
                in-order ACT queue (that cycle gates the whole pipeline)."""
                # partition p, block u=2t+k holds DRAM row i=256t+2p+k
                if gb == 0 and m == 0:
                    text_il = first_text
                else:
                    text_il = ptext.tile([128, NT * D], BF16, tag="text")
                    src = t_text[gb, m].rearrange(
                        "(t p k) d -> p t k d", p=128, k=2)
                    dst = text_il.rearrange(
                        "p (t k d) -> p t k d", t=NTH, k=2)
                    nsi = CFG["split_in"]
                    for h in range(nsi):
                        hh = NTH // nsi
                        getattr(nc, CFG["q_tin"]).dma_start(
                            out=dst[:, h * hh:(h + 1) * hh],
                            in_=src[:, h * hh:(h + 1) * hh])
                ttp = ps_ttp.tile([128, JX], BF16, tag="ttp")
                for u in range(NT):
                    nc.tensor.transpose(
                        ttp[:, u * 128:(u + 1) * 128],
                        text_il[:, u * D:(u + 1) * D], identb)
                textd = ptextd.tile([128, JX], BF16, tag="textd")
                # first units: DVE is idle during the ramp while the ACT
                # queue position gates the first cross->exp
                if gb * M + m < CFG["head_textd_dve"]:
                    nc.vector.tensor_copy(textd, ttp)
                elif CFG["eng_textd"] == "act":
                    nc.scalar.copy(out=textd, in_=ttp)
                else:
                    nc.vector.tensor_copy(textd, ttp)
                return text_il, textd

            def load_params(gb, queue=None, part=None):
                qd = getattr(nc, queue or CFG["q_small"])
                if part in (None, 0):
                    pk_sb = perb.tile([D, D + JQ + 3], BF16, tag="pk")
                    qd.dma_start(out=pk_sb, in_=t_pk[gb])
                    load_params.pk = pk_sb
                if part in (None, 1):
                    q2_sb = perb.tile([JQ, 1], F32, tag="q2")
                    qd.dma_start(out=q2_sb, in_=t_q2[gb])
                    load_params.q2 = q2_sb
                return load_params.pk, load_params.q2

            def taupart(tabu, text_il, etq_ap, rzt=None):
                """text_attn broadcast (every PE column = etq) + Zt bcast
                + reciprocal.  etq_ap is a [128, NT] AP of exp(qlmax)."""
                for u in range(NT):
                    nc.tensor.matmul(
                        tabu[:, 0:D],
                        _col_bcast(etq_ap[:, u:u + 1], 128),
                        text_il[:, u * D:(u + 1) * D],
                        start=(u == 0), stop=(u == NT - 1))
                if rzt is None:
                    for u in range(NT):
                        nc.tensor.matmul(
                            tabu[:, D:D + 1],
                            _col_bcast(etq_ap[:, u:u + 1], 128),
                            ones128[:, 0:1],
                            start=(u == 0), stop=(u == NT - 1))
                    rzt = psmall.tile([128, 1], F32, tag="rzt")
                    nc.vector.reciprocal(out=rzt, in_=tabu[:, D:D + 1])
                return rzt

            pending = [prep_text(0, 0)]
            prep_next = [1]
            pending_backend = None
            pair_stash = []
            etr_pair = None
            # gb0 params ride the idle SP queue at startup so the ACT
            # sequencer is free for the first textd copy
            params = load_params(0, queue=CFG["q_first"])
            for gb in range(BLOC):
                pk_sb, q2_sb = params
                # w1 is folded into every wq3 column on the host, so cross
                # already includes t1 and eT = exp(full logits); the per-i
                # t1 shift cancels in softmax_j and max_j eT = exp(qlmax)
                wq3_sb = pk_sb[:, 0:JQ]
                qn_sb = pk_sb[0:JQ, JQ: JQ + D + 1]

                for m in range(M):
                    # prefetch next batch's params mid-batch so the first
                    # unit of gb+1 never stalls on them; the two triggers go
                    # in different units so their sequencer cost spreads out
                    if gb + 1 < BLOC:
                        if m == 3:
                            load_params(gb + 1, part=0)
                        elif m == 5:
                            params = load_params(gb + 1, part=1)
                    unit = gb * M + m
                    text_il, textd = pending.pop(0)
                    text3 = text_il.rearrange("p (u d) -> p u d", d=D)

                    # ---- crossT_aug = [w3q|w1].T @ text_d  [65, 512] ----
                    cross = ps_cross.tile([JQ, JX], F32, tag="cross")
                    nc.tensor.matmul(cross, wq3_sb, textd, start=True, stop=True)

                    # ---- eT = exp(cross + q2) (row 64 = exp(t1)) ----
                    # split into column halves: same total ACT time (init is
                    # width-independent) but downstream consumers of half 0
                    # start ~400ns earlier
                    eT = pet.tile([JQ, JX], BF16, tag="eT")
                    nse = CFG["split_exp"]
                    he = JX // nse
                    for h in range(nse):
                        nc.scalar.activation(
                            out=eT[:, h * he:(h + 1) * he],
                            in_=cross[:, h * he:(h + 1) * he],
                            func=mybir.ActivationFunctionType.Exp,
                            bias=q2_sb[:, 0:1], scale=1.0)

                    # prep the NEXT unit now: its ACT copy lands after exp(n)
                    # and its PE transposes after cross(n), so neither blocks
                    # the cross->exp critical cycle.  Near the tail, prep
                    # additional units ahead so the final exps bunch early.
                    ahead = 1
                    if BLOC * M - unit <= CFG["tail_prep"]:
                        ahead = 2
                    while len(pending) < ahead and prep_next[0] < BLOC * M:
                        nxt = prep_next[0]
                        prep_next[0] += 1
                        pending.append(prep_text(nxt // M, nxt % M))

                    # ---- transpose eT slices -> etr [128, 4*64] ----
                    JB = JQ
                    if CFG["pair_ql"]:
                        if unit % 2 == 0:
                            etr_pair = ps_etr.tile(
                                [128, 2 * NT * JB], BF16, tag="etr")
                        etr = etr_pair[:, (unit % 2) * NT * JB:
                                       (unit % 2 + 1) * NT * JB]
                    else:
                        etr_t = ps_etr.tile([128, NT * JB], BF16, tag="etr")
                        etr = etr_t[:, :]
                    for u in range(NT):
                        nc.tensor.transpose(
                            etr[:, u * JB:u * JB + JQ],
                            eT[:, u * 128:(u + 1) * 128], identb64)

                    # ---- qlmax path: etq = exp(qlmax) = G * exp(t1) ----
                    # (pair mode: one reduce+mul covers TWO units' etr,
                    # halving the per-op DVE init overhead)
                    etq = etq_pair = None
                    if not CFG["pair_ql"]:
                        etr_blk = etr.rearrange("p (u j) -> p u j", j=JB)
                        etq = psmall.tile([128, NT], BF16, tag="etq")
                        nc.vector.tensor_reduce(
                            out=etq, in_=etr_blk,
                            axis=mybir.AxisListType.X, op=mybir.AluOpType.max)
                    elif unit % 2 == 1:
                        pblk = etr_pair[:, :].rearrange(
                            "p (w j) -> p w j", j=JB)
                        etq_pair = psmall.tile([128, 2 * NT], BF16, tag="etq")
                        nc.vector.tensor_reduce(
                            out=etq_pair, in_=pblk,
                            axis=mybir.AxisListType.X, op=mybir.AluOpType.max)

                    # ---- attnu = eT[0:64].T @ qn; qa = attnu*rq ----
                    # tabu regions: [0:D] text_attn bcast, [D] Zt bcast,
                    # [D+1:D+1+NT] attnu softmax denominators Z(i)
                    tabu = ps_tabc.tile([128, D + 1 + NT], F32, tag="tabu")
                    o123 = po123.tile([128, NT, 3 * D], FP16, tag="o123")
                    if CFG["qa_merge"]:
                        onesq = pk_sb[0:JQ, JQ + D:JQ + 1 + D]
                        attnu = ps_attnu.tile([128, NT * D], F32, tag="attnu")
                        for u in range(NT):
                            nc.tensor.matmul(
                                attnu[:, u * D:(u + 1) * D],
                                eT[0:JQ, u * 128:(u + 1) * 128],
                                qn_sb[:, 0:D], start=True, stop=True)
                        for u in range(NT):
                            nc.tensor.matmul(
                                tabu[:, D + 1 + u:D + 2 + u],
                                eT[0:JQ, u * 128:(u + 1) * 128],
                                onesq, start=True, stop=True)
                        if CFG["merge_recip"]:
                            # Zt via accumulating 1-column matmuls (etq col
                            # bcast stationary, ones col moving) issued here
                            # so ONE reciprocal covers [Zt | Z(i) x4]
                            for u in range(NT):
                                nc.tensor.matmul(
                                    tabu[:, D:D + 1],
                                    _col_bcast(etq[:, u:u + 1], 128),
                                    ones128[:, 0:1],
                                    start=(u == 0), stop=(u == NT - 1))
                            rqz = psmall.tile([128, NT + 1], F32, tag="rqz")
                            nc.vector.reciprocal(
                                out=rqz, in_=tabu[:, D:D + 1 + NT])
                            rq = rqz[:, 1:NT + 1]
                            rzt = rqz[:, 0:1]
                        else:
                            rq = psmall.tile([128, NT], F32, tag="rq")
                            nc.vector.reciprocal(
                                out=rq, in_=tabu[:, D + 1:D + 1 + NT])
                            rzt = None
                        if BLOC * M - unit <= CFG["tail_qa_act"]:
                            for u in range(NT):
                                nc.scalar.mul(
                                    out=o123[:, u, 0:D],
                                    in_=attnu[:, u * D:(u + 1) * D],
                                    mul=rq[:, u:u + 1])
                        else:
                            nc.vector.tensor_tensor(
                                out=o123[:, :, 0:D],
                                in0=attnu.rearrange("p (u d) -> p u d", d=D),
                                in1=_bcast(rq, D, 2),
                                op=mybir.AluOpType.mult)
                    else:
                        raise NotImplementedError("qa_merge=False removed")

                    if not CFG["pair_ql"]:
                        rzt = taupart(tabu, text_il, etq, rzt)
                    # ---- col2 = text*qa (qa is fresh; stays in-unit) ----
                    tail = BLOC * M - unit <= CFG["tail_cols_dve"]
                    dve_u = NT if tail else NT - CFG["col2_pool_u"]
                    spans = (((nc.vector, 0, 2), (nc.gpsimd, 2, NT)) if tail
                             else ((nc.vector, 0, dve_u),
                                   (nc.gpsimd, dve_u, NT)))
                    for eng, u0, u1 in spans:
                        if u1 <= u0:
                            continue
                        eng.tensor_mul(
                            o123[:, u0:u1, D:2 * D],
                            text3[:, u0:u1, :], o123[:, u0:u1, 0:D])

                    def backend(rzt, gb=gb, m=m, unit=unit, tabu=tabu,
                                o123=o123, text3=text3):
                        """tabcN + col3 + store; optionally run one unit
                        late so the ACT tabcN never head-blocks on a fresh
                        rzt and the store order stays compute-paced."""
                        tabc = ptabc.tile([128, D], BF16, tag="tabc")
                        if CFG["eng_tabc"] == "act":
                            nc.scalar.mul(out=tabc, in_=tabu[:, 0:D],
                                          mul=rzt)
                        else:
                            nc.vector.tensor_scalar_mul(
                                out=tabc, in0=tabu[:, 0:D], scalar1=rzt)
                        tail = BLOC * M - unit <= CFG["tail_cols_dve"]
                        dve_u = NT if tail else NT - CFG["col3_pool_u"]
                        spans = (((nc.vector, 0, 2), (nc.gpsimd, 2, NT))
                                 if tail else ((nc.vector, 0, dve_u),
                                               (nc.gpsimd, dve_u, NT)))
                        for eng, u0, u1 in spans:
                            if u1 <= u0:
                                continue
                            eng.tensor_mul(
                                o123[:, u0:u1, 2 * D:3 * D],
                                text3[:, u0:u1, :],
                                _bcast(tabc[:, :], u1 - u0, 1))
                        nsp = CFG["split_out"]
                        if BLOC * M - unit <= CFG["tail_split"]:
                            nsp = max(nsp, 2)
                        ht = NT // nsp
                        dst4 = t_out[gb, m].rearrange(
                            "(t p k) c -> p t k c", p=128, k=2)
                        o1234 = o123[:, :, :].rearrange(
                            "p (t k) c -> p t k c", k=2)
                        for h in range(nsp):
                            ts0, ts1 = h * ht, (h + 1) * ht
                            getattr(nc, CFG["q_out"]).dma_start(
                                out=dst4[:, ts0 // 2:ts1 // 2],
                                in_=o1234[:, ts0 // 2:ts1 // 2])

                    if CFG["pair_ql"]:
                        pair_stash.append((tabu, text_il, backend))
                        if unit % 2 == 1:
                            for i, (tb, til, bk) in enumerate(pair_stash):
                                bk(taupart(
                                    tb, til,
                                    etq_pair[:, i * NT:(i + 1) * NT]))
                            pair_stash = []
                    elif (CFG["defer_ta"]
                            and BLOC * M - unit > CFG["defer_tail_off"]):
                        if pending_backend is not None:
                            pending_backend()
                        pending_backend = (lambda r=rzt, b=backend: b(r))
                    else:
                        if pending_backend is not None:
                            pending_backend()
                            pending_backend = None
                        backend(rzt)

            if pending_backend is not None:
                pending_backend()

    _split_multi_waits(nc)
    return nc


_NC_CACHE = {}


def _get_nc():
    if "nc" not in _NC_CACHE:
        _NC_CACHE["nc"] = _build_program()
    return _NC_CACHE["nc"]


def _make_in_maps(text, query, w):
    w1, w2, w3 = w[:D], w[D:2 * D], w[2 * D:]
    in_maps = []
    for c in range(NCORES):
        sl = slice(c * BLOC, (c + 1) * BLOC)
        q = query[sl]                                    # [BLOC, 64, 128]
        q2 = np.einsum("bjd,d->bj", q, w2)[:, :, None]
        # packed [D, 64 + 129 + 2]: [0:64] = w3*q_j + w1 (w1 folded into
        # every column so cross includes t1); rows 0:64 of [64:193] =
        # [qn | ones]; trailing cols pad
        pk = np.zeros((BLOC, D, D + JQ + 3), np.float32)
        pk[:, :, 0:JQ] = (np.einsum("bjd->bdj", q * w3[None, None, :])
                          + w1[None, :, None])
        pk[:, 0:JQ, JQ:JQ + D] = q
        pk[:, 0:JQ, JQ + D] = 1.0
        m = {
            "text": np.ascontiguousarray(text[sl]).astype(ml_dtypes.bfloat16),
            "packed": np.ascontiguousarray(pk).astype(ml_dtypes.bfloat16),
            "q2aug": np.ascontiguousarray(q2, dtype=np.float32),
        }
        in_maps.append(m)
    return in_maps


def kernel(text, query, text_mask, query_mask, w, b, _want_results=False):
    text = np.asarray(text, dtype=np.float32)
    query = np.asarray(query, dtype=np.float32)
    w = np.asarray(w, dtype=np.float32)
    nc = _get_nc()
    in_maps = _make_in_maps(text, query, w)
    res = run_bass_kernel_spmd(nc, in_maps, core_ids=list(range(NCORES)))
    out = np.empty((B, M, JX, 4 * D), dtype=np.float32)
    out[..., 0:D] = text
    for c in range(NCORES):
        out[c * BLOC:(c + 1) * BLOC, ..., D:] = res.results[c]["out"]
    if _want_results:
        return out, res
    return out
